# revision 12
# baseline (speedup 1.0000x reference)
"""Bilinear interpolation (spatial transformer) Trainium2 kernel.

Strategy (pure data parallel, 8 images per NeuronCore):
  Per image, build J4 in DRAM: J4[y*384+x] = 12 f32 = the 4 bilinear corner
  pixels [I(y,x,:), I(y,x+1,:), I(y+1,x,:), I(y+1,x+1,:)] (clamped at edges;
  out-of-range halves are zero-weighted by the bilinear weights, matching the
  reference's clip arithmetic exactly).
  Per output pixel, compute the affine sample position, floor/clip (exact
  reference arithmetic), the 4 bilinear weights, and a J4 slot index; gather
  48B per pixel via indirect SWDGE DMA (one instruction per output-pixel
  column of a 128x384 tile = 128 descriptors each; the HW only honors one
  offset per partition per instruction, so this cannot be batched further).
  Out-of-domain pixels (whose 4 bilinear weights are all exactly zero) get
  slot index >= 2^20 and are skipped by the DMA bounds check, which skips
  their HBM reads. Weighted-combine on DVE; store.

  Host side: images are assigned to cores by greedy balancing of the
  estimated in-bounds sample count.

  Perf notes (measured on TRN2 via axon): the kernel is bound by the
  indirect-DMA descriptor-slot rate (~14 ns/slot; 1.18M slots/core), which
  is insensitive to payload size (48B..512B), bounds-check skips, and
  SWDGE queue count. dma_gather (256B blocks) measures ~22 ns/desc, GPSIMD
  ap_gather ~3 G elem/s - both slower. Per-pixel gathering is the floor.

Processing tiles: "thirds" of an image = 128 output rows, one row per
partition, 384 pixels along the free dim.
"""
import sys

sys.path.insert(0, "/opt/trn_rl_repo")

import numpy as np

H = 384
W = 384
C = 3
B = 64
NCORES = 8
BPC = B // NCORES          # images per core
NT = 3                     # thirds per image
ROWS_T = 128               # output rows per third
IMG_ELEMS = H * W * C      # 442368
ROW_ELEMS = W * C          # 1152
NSLOT = H * W              # J4 slots per image
BIG = float(1 << 20)       # OOB slot sentinel (> NSLOT-1 -> descriptor skipped)

_CACHE = {}


def _build_program():
    import concourse.bass as bass
    import concourse.bacc as bacc
    import concourse.mybir as mybir
    from concourse import tile

    f32 = mybir.dt.float32
    i32 = mybir.dt.int32
    AF = mybir.ActivationFunctionType
    OP = mybir.AluOpType

    nc = bacc.Bacc("TRN2", target_bir_lowering=False, debug=False,
                   num_devices=NCORES)

    images = nc.dram_tensor("images", [BPC * IMG_ELEMS + 1280], f32, kind="ExternalInput")
    theta = nc.dram_tensor("theta", [1, BPC * 6], f32, kind="ExternalInput")
    xs_c = nc.dram_tensor("xs_c", [128, W], f32, kind="ExternalInput")
    ys_c = nc.dram_tensor("ys_c", [128, NT], f32, kind="ExternalInput")
    out_d = nc.dram_tensor("out", [BPC, IMG_ELEMS], f32, kind="ExternalOutput")

    def dram_ap(t, off, layout):
        return bass.AP(t, off, layout)

    # Single TileContext. J4 lives in a 2-slot DRAM tile ring so Tile tracks
    # the store->gather RAW and gather->rebuild WAR dependencies, letting the
    # J4 build of image m+1 overlap the gathers of image m.
    with tile.TileContext(nc) as tc:
        with (
            tc.tile_pool(name="jbuild", bufs=2) as jpool,
            tc.tile_pool(name="j4d", bufs=3, space="DRAM") as j4pool,
            tc.tile_pool(name="consts", bufs=1) as cpool,
            tc.tile_pool(name="work", bufs=2) as wpool,
            tc.tile_pool(name="gather", bufs=2) as gpool,
        ):
            # constants
            xs_t = cpool.tile([128, W], f32, tag="xs")
            nc.sync.dma_start(xs_t[:], xs_c[:])
            ys_t = cpool.tile([128, NT], f32, tag="ys")
            nc.sync.dma_start(ys_t[:], ys_c[:])
            th_row = cpool.tile([1, BPC * 6], f32, tag="throw")
            nc.sync.dma_start(th_row[:], theta[:])
            th = cpool.tile([128, BPC * 6], f32, tag="th")
            nc.gpsimd.partition_broadcast(th[:], th_row[:])

            # zero the two gather ring buffers once: skipped (OOB) slots keep
            # stale contents, which must be finite so that weight*value == 0.
            for _gi in range(2):
                gz = gpool.tile([128, W * 12], f32, tag="g", name=f"gz{_gi}")
                nc.vector.memset(gz[:], 0.0)

            # ------------- per image: build J4, then sample+gather -------------
            for m in range(BPC):
                base = m * IMG_ELEMS
                j4m = j4pool.tile([NSLOT, 12], f32, tag="j4", name=f"j4m{m}")
                for T in range(NT):
                    im0 = jpool.tile([128, ROW_ELEMS + 8], f32, tag="im0")
                    im1 = jpool.tile([128, ROW_ELEMS + 8], f32, tag="im1")
                    # rows 128T+p  (pad +8 reads into the next row / next image;
                    # images tensor is padded so this never faults)
                    off0 = base + T * 128 * ROW_ELEMS
                    nc.sync.dma_start(
                        im0[:],
                        dram_ap(images, off0, [[ROW_ELEMS, 128], [1, ROW_ELEMS + 8]]),
                    )
                    # rows 128T+p+1. For T==2, partition 127 reads "row 384",
                    # i.e. past the image: next image's row 0, or the zero pad
                    # for the last image. Those taps are zero-weighted
                    # (y0==383 implies all four bilinear weights are 0), so
                    # any finite values are fine.
                    nc.sync.dma_start(
                        im1[:],
                        dram_ap(images, off0 + ROW_ELEMS,
                                [[ROW_ELEMS, 128], [1, ROW_ELEMS + 8]]),
                    )

                    j4t = jpool.tile([128, W * 12], f32, tag="j4t")
                    # slot layout per partition: [x: 384][tap: 4][c: 3]
                    o3 = j4t[:].rearrange("p (x t c) -> p x t c", t=4, c=3)

                    def xc3(t_, off):
                        return t_[:, off : off + W * 3].rearrange("p (x c) -> p x c", c=3)

                    # taps: 0=(y,x) 1=(y,x+1) 2=(y+1,x) 3=(y+1,x+1)
                    nc.scalar.copy(o3[:, :, 0, :], xc3(im0, 0))
                    nc.scalar.copy(o3[:, :, 1, :], xc3(im0, 3))
                    nc.vector.tensor_copy(o3[:, :, 2, :], xc3(im1, 0))
                    nc.vector.tensor_copy(o3[:, :, 3, :], xc3(im1, 3))

                    nc.sync.dma_start(
                        j4m[T * 128 * W : (T + 1) * 128 * W, :]
                        .rearrange("(p q) c -> p (q c)", p=128),
                        j4t[:],
                    )

                t00 = th[:, m * 6 + 0 : m * 6 + 1]
                t01 = th[:, m * 6 + 1 : m * 6 + 2]
                t02 = th[:, m * 6 + 2 : m * 6 + 3]
                t10 = th[:, m * 6 + 3 : m * 6 + 4]
                t11 = th[:, m * 6 + 4 : m * 6 + 5]
                t12 = th[:, m * 6 + 5 : m * 6 + 6]
                for T in range(NT):
                    ysT = ys_t[:, T : T + 1]
                    tiny = wpool.tile([128, 2], f32, tag="tiny")
                    # ys*t01, ys*t11
                    nc.vector.tensor_scalar(tiny[:, 0:1], ysT, t01, None, OP.mult)
                    nc.vector.tensor_scalar(tiny[:, 1:2], ysT, t11, None, OP.mult)

                    x = wpool.tile([128, W], f32, tag="x")
                    y = wpool.tile([128, W], f32, tag="y")
                    nc.vector.tensor_scalar(x[:], xs_t[:], t00, None, OP.mult)
                    nc.vector.tensor_scalar(x[:], x[:], tiny[:, 0:1], t02, OP.add, OP.add)
                    nc.vector.tensor_scalar(x[:], x[:], 1.0, float(W) * 0.5, OP.add, OP.mult)
                    nc.vector.tensor_scalar(y[:], xs_t[:], t10, None, OP.mult)
                    nc.vector.tensor_scalar(y[:], y[:], tiny[:, 1:2], t12, OP.add, OP.add)
                    nc.vector.tensor_scalar(y[:], y[:], 1.0, float(H) * 0.5, OP.add, OP.mult)

                    # floors (round-to-nearest-even of v-0.5 == floor except at
                    # exact integers, where bilinear output is unaffected)
                    x0i = wpool.tile([128, W], i32, tag="x0i")
                    y0i = wpool.tile([128, W], i32, tag="y0i")
                    nc.scalar.activation(x0i[:], x[:], AF.Copy, bias=-0.5)
                    nc.scalar.activation(y0i[:], y[:], AF.Copy, bias=-0.5)
                    x0f = wpool.tile([128, W], f32, tag="x0f")
                    y0f = wpool.tile([128, W], f32, tag="y0f")
                    nc.scalar.activation(x0f[:], x0i[:], AF.Copy)
                    nc.scalar.activation(y0f[:], y0i[:], AF.Copy)

                    x0c = wpool.tile([128, W], f32, tag="x0c")
                    x1c = wpool.tile([128, W], f32, tag="x1c")
                    y0c = wpool.tile([128, W], f32, tag="y0c")
                    y1c = wpool.tile([128, W], f32, tag="y1c")
                    nc.vector.tensor_scalar(x0c[:], x0f[:], 0.0, float(W - 1), OP.max, OP.min)
                    nc.vector.tensor_scalar(x1c[:], x0f[:], -1.0, 1.0, OP.max, OP.add)
                    nc.vector.tensor_scalar(x1c[:], x1c[:], float(W - 1), None, OP.min)
                    nc.vector.tensor_scalar(y0c[:], y0f[:], 0.0, float(H - 1), OP.max, OP.min)
                    nc.vector.tensor_scalar(y1c[:], y0f[:], -1.0, 1.0, OP.max, OP.add)
                    nc.vector.tensor_scalar(y1c[:], y1c[:], float(H - 1), None, OP.min)

                    xc = wpool.tile([128, W], f32, tag="xc")
                    yc = wpool.tile([128, W], f32, tag="yc")
                    nc.vector.tensor_scalar(xc[:], x[:], 0.0, float(W - 1), OP.max, OP.min)
                    nc.vector.tensor_scalar(yc[:], y[:], 0.0, float(H - 1), OP.max, OP.min)

                    dxa = wpool.tile([128, W], f32, tag="dxa")
                    dxc = wpool.tile([128, W], f32, tag="dxc")
                    dya = wpool.tile([128, W], f32, tag="dya")
                    dyb = wpool.tile([128, W], f32, tag="dyb")
                    nc.vector.tensor_tensor(dxa[:], x1c[:], xc[:], OP.subtract)
                    nc.vector.tensor_tensor(dxc[:], xc[:], x0c[:], OP.subtract)
                    nc.vector.tensor_tensor(dya[:], y1c[:], yc[:], OP.subtract)
                    nc.vector.tensor_tensor(dyb[:], yc[:], y0c[:], OP.subtract)

                    wa = wpool.tile([128, W], f32, tag="wa")
                    wb = wpool.tile([128, W], f32, tag="wb")
                    wc_ = wpool.tile([128, W], f32, tag="wc")
                    wd = wpool.tile([128, W], f32, tag="wd")
                    nc.vector.tensor_tensor(wa[:], dxa[:], dya[:], OP.mult)
                    nc.vector.tensor_tensor(wb[:], dxa[:], dyb[:], OP.mult)
                    nc.vector.tensor_tensor(wc_[:], dxc[:], dya[:], OP.mult)
                    nc.vector.tensor_tensor(wd[:], dxc[:], dyb[:], OP.mult)

                    # J4 slot index; OOB pixels (all-zero weights) get +BIG so
                    # the gather's bounds check skips their descriptors.
                    vf = wpool.tile([128, W], f32, tag="vf")
                    nc.vector.scalar_tensor_tensor(
                        vf[:], y0c[:], float(W), x0c[:], op0=OP.mult, op1=OP.add
                    )
                    # u = x0f*(382-x0f) >= 0 iff 0 <= x0 <= 382; same for y
                    t1 = wpool.tile([128, W], f32, tag="t1")
                    u = wpool.tile([128, W], f32, tag="u")
                    nc.vector.tensor_scalar(t1[:], x0f[:], float(W - 2), None, OP.subtract)
                    nc.vector.scalar_tensor_tensor(u[:], x0f[:], -1.0, t1[:], op0=OP.mult, op1=OP.mult)
                    t2 = wpool.tile([128, W], f32, tag="t2")
                    v = wpool.tile([128, W], f32, tag="v")
                    nc.vector.tensor_scalar(t2[:], y0f[:], float(H - 2), None, OP.subtract)
                    nc.vector.scalar_tensor_tensor(v[:], y0f[:], -1.0, t2[:], op0=OP.mult, op1=OP.mult)
                    wmin = wpool.tile([128, W], f32, tag="wmin")
                    nc.vector.tensor_tensor(wmin[:], u[:], v[:], OP.min)
                    s = wpool.tile([128, W], f32, tag="s")
                    nc.vector.tensor_scalar(s[:], wmin[:], 0.0, None, OP.is_lt)
                    vf2 = wpool.tile([128, W], f32, tag="vf2")
                    nc.vector.scalar_tensor_tensor(vf2[:], s[:], BIG, vf[:], op0=OP.mult, op1=OP.add)

                    vi = wpool.tile([128, W], i32, tag="vi")
                    nc.vector.tensor_copy(vi[:], vf2[:])

                    g = gpool.tile([128, W * 12], f32, tag="g")
                    for k in range(W):
                        nc.gpsimd.indirect_dma_start(
                            out=g[:, k * 12 : (k + 1) * 12],
                            out_offset=None,
                            in_=j4m[:],
                            in_offset=bass.IndirectOffsetOnAxis(ap=vi[:, k : k + 1], axis=0),
                            bounds_check=NSLOT - 1,
                            oob_is_err=False,
                        )

                    def wbc(t_):
                        return t_[:].rearrange("p (k one) -> p k one", one=1).to_broadcast([128, W, 3])

                    acc = wpool.tile([128, W * 3], f32, tag="acc")
                    tmp = wpool.tile([128, W * 3], f32, tag="tmp")
                    a3 = acc[:].rearrange("p (k c) -> p k c", c=3)
                    t3 = tmp[:].rearrange("p (k c) -> p k c", c=3)
                    g4 = g[:].rearrange("p (k t c) -> p k t c", t=4, c=3)
                    nc.vector.tensor_tensor(a3[:], g4[:, :, 0, :], wbc(wa), OP.mult)
                    nc.vector.tensor_tensor(t3[:], g4[:, :, 1, :], wbc(wc_), OP.mult)
                    nc.vector.tensor_tensor(a3[:], a3[:], t3[:], OP.add)
                    nc.vector.tensor_tensor(t3[:], g4[:, :, 2, :], wbc(wb), OP.mult)
                    nc.vector.tensor_tensor(a3[:], a3[:], t3[:], OP.add)
                    nc.vector.tensor_tensor(t3[:], g4[:, :, 3, :], wbc(wd), OP.mult)
                    nc.vector.tensor_tensor(a3[:], a3[:], t3[:], OP.add)

                    nc.sync.dma_start(
                        dram_ap(out_d, m * IMG_ELEMS + T * 128 * ROW_ELEMS,
                                [[ROW_ELEMS, 128], [1, ROW_ELEMS]]),
                        acc[:],
                    )

    nc.compile()
    return nc


class _Runner:
    def __init__(self, nc, n_cores):
        import jax
        from jax.sharding import Mesh, PartitionSpec
        from jax.experimental.shard_map import shard_map
        import concourse.mybir as mybir
        from concourse.bass2jax import (
            _bass_exec_p, partition_id_tensor, install_neuronx_cc_hook,
        )

        install_neuronx_cc_hook()
        self.jax = jax
        self.n_cores = n_cores
        partition_name = nc.partition_id_tensor.name if nc.partition_id_tensor else None
        in_names, out_names, out_avals, zero_outs = [], [], [], []
        for alloc in nc.m.functions[0].allocations:
            if not isinstance(alloc, mybir.MemoryLocationSet):
                continue
            name = alloc.memorylocations[0].name
            if alloc.kind == "ExternalInput":
                if name != partition_name:
                    in_names.append(name)
            elif alloc.kind == "ExternalOutput":
                shape = tuple(alloc.tensor_shape)
                dtype = mybir.dt.np(alloc.dtype)
                out_avals.append(jax.core.ShapedArray(shape, dtype))
                out_names.append(name)
                zero_outs.append(np.zeros(shape, dtype))
        self.in_names = list(in_names)
        self.out_names = out_names
        self.zero_outs = zero_outs
        n_params = len(in_names)
        n_outs = len(out_names)
        all_in_names = in_names + out_names
        if partition_name is not None:
            all_in_names.append(partition_name)

        def _body(*args):
            operands = list(args)
            if partition_name is not None:
                operands.append(partition_id_tensor())
            outs = _bass_exec_p.bind(
                *operands,
                out_avals=tuple(out_avals),
                in_names=tuple(all_in_names),
                out_names=tuple(out_names),
                lowering_input_output_aliases=(),
                sim_require_finite=False,
                sim_require_nnan=False,
                nc=nc,
            )
            return tuple(outs)

        devices = jax.devices()[:n_cores]
        self.mesh = Mesh(np.asarray(devices), ("core",))
        in_specs = (PartitionSpec("core"),) * (n_params + n_outs)
        out_specs = (PartitionSpec("core"),) * n_outs
        self.fn = jax.jit(
            shard_map(_body, mesh=self.mesh, in_specs=in_specs,
                      out_specs=out_specs, check_rep=False),
            keep_unused=True,
        )

    def run(self, in_maps):
        from jax.sharding import NamedSharding, PartitionSpec
        sharding = NamedSharding(self.mesh, PartitionSpec("core"))
        concat = [
            np.concatenate([np.asarray(m[name]) for m in in_maps], axis=0)
            for name in self.in_names
        ]
        concat += [
            np.zeros((self.n_cores * z.shape[0], *z.shape[1:]), z.dtype)
            for z in self.zero_outs
        ]
        args = [self.jax.device_put(a, sharding) for a in concat]
        outs = self.fn(*args)
        self.jax.block_until_ready(outs)
        res = []
        for c in range(self.n_cores):
            d = {}
            for i, name in enumerate(self.out_names):
                a = np.asarray(outs[i])
                per_core = (self.n_cores, a.shape[0] // self.n_cores) + a.shape[1:]
                d[name] = a.reshape(per_core)[c]
            res.append(d)
        return res


def _get_runner():
    if "runner" not in _CACHE:
        nc = _build_program()
        _CACHE["runner"] = _Runner(nc, NCORES)
    return _CACHE["runner"]


def _host_constants():
    import jax.numpy as jnp

    xs = np.asarray(jnp.linspace(-1.0, 1.0, W, dtype=jnp.float32))
    ys = np.asarray(jnp.linspace(-1.0, 1.0, H, dtype=jnp.float32))
    xs_c = np.tile(xs[None, :], (128, 1)).astype(np.float32)
    # ys_c[p, T] = ys[128*T + p]
    ys_c = ys.reshape(NT, 128).T.copy().astype(np.float32)
    return xs_c, ys_c


def _assign_images(theta):
    """Greedy balance of per-image in-bounds sample counts across cores.

    The gather skips descriptors for out-of-bounds pixels, so per-image
    device cost is proportional to its in-bounds fraction. Estimate it on a
    coarse 64x64 grid and pack images into cores (8 each) greedily.
    Returns perm: perm[c*BPC+s] = original image index placed at core c slot s.
    """
    g = np.linspace(-1.0, 1.0, 64, dtype=np.float32)
    xx, yy = np.meshgrid(g, g)
    coords = np.stack([xx.ravel(), yy.ravel(), np.ones(xx.size, np.float32)], 0)
    t = theta.astype(np.float32) @ coords          # [B, 2, n]
    x = (t[:, 0] + 1.0) * (W * 0.5)
    y = (t[:, 1] + 1.0) * (H * 0.5)
    x0 = np.floor(x)
    y0 = np.floor(y)
    cnt = ((x0 >= 0) & (x0 <= W - 2) & (y0 >= 0) & (y0 <= H - 2)).sum(1)
    order = np.argsort(-cnt, kind="stable")
    loads = np.zeros(NCORES)
    slots = [[] for _ in range(NCORES)]
    for i in order:
        free = [c for c in range(NCORES) if len(slots[c]) < BPC]
        c = min(free, key=lambda c: loads[c])
        slots[c].append(int(i))
        loads[c] += cnt[i]
    return np.array([i for c in range(NCORES) for i in slots[c]], dtype=np.int64)


def build_in_maps(images, theta):
    """Shared by kernel() and any external timing harness."""
    images = np.ascontiguousarray(images, dtype=np.float32)
    theta = np.ascontiguousarray(theta, dtype=np.float32)
    perm = _assign_images(theta)
    xs_c, ys_c = _host_constants()
    in_maps = []
    for c in range(NCORES):
        idx = perm[c * BPC : (c + 1) * BPC]
        imgs = images[idx].reshape(-1)
        imgs = np.concatenate([imgs, np.zeros(1280, np.float32)])
        th = theta[idx].reshape(1, BPC * 6)
        in_maps.append({"images": imgs, "theta": th, "xs_c": xs_c, "ys_c": ys_c})
    return in_maps, perm


def kernel(images, theta):
    images = np.ascontiguousarray(images, dtype=np.float32)
    theta = np.ascontiguousarray(theta, dtype=np.float32)
    assert images.shape == (B, H, W, C) and theta.shape == (B, 2, 3)
    runner = _get_runner()
    in_maps, perm = build_in_maps(images, theta)
    res = runner.run(in_maps)
    out = np.empty((B, H, W, C), np.float32)
    for c in range(NCORES):
        out[perm[c * BPC : (c + 1) * BPC]] = res[c]["out"].reshape(BPC, H, W, C)
    return out


# revision 13
# speedup vs baseline: 1.1501x; 1.1501x over previous
"""Bilinear interpolation (spatial transformer) Trainium2 kernel.

Strategy (pure data parallel, 8 images per NeuronCore):
  Per image, build J4 in DRAM: J4[y*384+x] = 12 f32 = the 4 bilinear corner
  pixels [I(y,x,:), I(y,x+1,:), I(y+1,x,:), I(y+1,x+1,:)] (clamped at edges;
  out-of-range halves are zero-weighted by the bilinear weights, matching the
  reference's clip arithmetic exactly).
  Per output pixel, compute the affine sample position, floor/clip (exact
  reference arithmetic), the 4 bilinear weights, and a J4 slot index; gather
  48B per pixel via indirect SWDGE DMA (one instruction per output-pixel
  column of a 128x384 tile = 128 descriptors each; the HW only honors one
  offset per partition per instruction, so this cannot be batched further).
  Out-of-domain pixels (whose 4 bilinear weights are all exactly zero) get
  slot index >= 2^20 and are skipped by the DMA bounds check, which skips
  their HBM reads. Weighted-combine on DVE; store.

  Host side: images are assigned to cores by greedy balancing of the
  estimated in-bounds sample count.

  Perf notes (measured on TRN2 via axon): the kernel is bound by the
  indirect-DMA descriptor-slot rate (~14 ns/slot; 1.18M slots/core), which
  is insensitive to payload size (48B..512B), bounds-check skips, and
  SWDGE queue count. dma_gather (256B blocks) measures ~22 ns/desc, GPSIMD
  ap_gather ~3 G elem/s - both slower. Per-pixel gathering is the floor.

Processing tiles: "thirds" of an image = 128 output rows, one row per
partition, 384 pixels along the free dim.
"""
import sys

sys.path.insert(0, "/opt/trn_rl_repo")

import numpy as np

H = 384
W = 384
C = 3
B = 64
NCORES = 8
BPC = B // NCORES          # images per core
NT = 3                     # thirds per image
ROWS_T = 128               # output rows per third
IMG_ELEMS = H * W * C      # 442368
ROW_ELEMS = W * C          # 1152
NSLOT = H * W              # J4 slots per image
BIG = float(1 << 20)       # OOB slot sentinel (> NSLOT-1 -> descriptor skipped)

_CACHE = {}


def _build_program():
    import concourse.bass as bass
    import concourse.bacc as bacc
    import concourse.mybir as mybir
    from concourse import tile

    f32 = mybir.dt.float32
    i32 = mybir.dt.int32
    AF = mybir.ActivationFunctionType
    OP = mybir.AluOpType

    nc = bacc.Bacc("TRN2", target_bir_lowering=False, debug=False,
                   num_devices=NCORES)

    images = nc.dram_tensor("images", [BPC * IMG_ELEMS + 1280], f32, kind="ExternalInput")
    theta = nc.dram_tensor("theta", [1, BPC * 6], f32, kind="ExternalInput")
    xs_c = nc.dram_tensor("xs_c", [128, W], f32, kind="ExternalInput")
    ys_c = nc.dram_tensor("ys_c", [128, NT], f32, kind="ExternalInput")
    out_d = nc.dram_tensor("out", [BPC, IMG_ELEMS], f32, kind="ExternalOutput")

    def dram_ap(t, off, layout):
        return bass.AP(t, off, layout)

    # Single TileContext. J4 lives in a 2-slot DRAM tile ring so Tile tracks
    # the store->gather RAW and gather->rebuild WAR dependencies, letting the
    # J4 build of image m+1 overlap the gathers of image m.
    with tile.TileContext(nc) as tc:
        with (
            tc.tile_pool(name="jbuild", bufs=2) as jpool,
            tc.tile_pool(name="j4d", bufs=2, space="DRAM") as j4pool,
            tc.tile_pool(name="consts", bufs=1) as cpool,
            tc.tile_pool(name="work", bufs=2) as wpool,
            tc.tile_pool(name="gather", bufs=2) as gpool,
        ):
            # constants
            xs_t = cpool.tile([128, W], f32, tag="xs")
            nc.sync.dma_start(xs_t[:], xs_c[:])
            ys_t = cpool.tile([128, NT], f32, tag="ys")
            nc.sync.dma_start(ys_t[:], ys_c[:])
            th_row = cpool.tile([1, BPC * 6], f32, tag="throw")
            nc.sync.dma_start(th_row[:], theta[:])
            th = cpool.tile([128, BPC * 6], f32, tag="th")
            nc.gpsimd.partition_broadcast(th[:], th_row[:])

            # zero the two gather ring buffers once: skipped (OOB) slots keep
            # stale contents, which must be finite so that weight*value == 0.
            for _gi in range(2):
                gz = gpool.tile([128, W * 12], f32, tag="g", name=f"gz{_gi}")
                nc.vector.memset(gz[:], 0.0)

            # ------------- per image: build J4, then sample+gather -------------
            for m in range(BPC):
                base = m * IMG_ELEMS
                j4m = j4pool.tile([NSLOT, 12], f32, tag="j4", name=f"j4m{m}")
                for T in range(NT):
                    im0 = jpool.tile([128, ROW_ELEMS + 8], f32, tag="im0")
                    im1 = jpool.tile([128, ROW_ELEMS + 8], f32, tag="im1")
                    # rows 128T+p  (pad +8 reads into the next row / next image;
                    # images tensor is padded so this never faults)
                    off0 = base + T * 128 * ROW_ELEMS
                    nc.sync.dma_start(
                        im0[:],
                        dram_ap(images, off0, [[ROW_ELEMS, 128], [1, ROW_ELEMS + 8]]),
                    )
                    # rows 128T+p+1. For T==2, partition 127 reads "row 384",
                    # i.e. past the image: next image's row 0, or the zero pad
                    # for the last image. Those taps are zero-weighted
                    # (y0==383 implies all four bilinear weights are 0), so
                    # any finite values are fine.
                    nc.sync.dma_start(
                        im1[:],
                        dram_ap(images, off0 + ROW_ELEMS,
                                [[ROW_ELEMS, 128], [1, ROW_ELEMS + 8]]),
                    )

                    j4t = jpool.tile([128, W * 12], f32, tag="j4t")
                    # slot layout per partition: [x: 384][tap: 4][c: 3]
                    o3 = j4t[:].rearrange("p (x t c) -> p x t c", t=4, c=3)

                    def xc3(t_, off):
                        return t_[:, off : off + W * 3].rearrange("p (x c) -> p x c", c=3)

                    # taps: 0=(y,x) 1=(y,x+1) 2=(y+1,x) 3=(y+1,x+1)
                    nc.scalar.copy(o3[:, :, 0, :], xc3(im0, 0))
                    nc.scalar.copy(o3[:, :, 1, :], xc3(im0, 3))
                    nc.vector.tensor_copy(o3[:, :, 2, :], xc3(im1, 0))
                    nc.vector.tensor_copy(o3[:, :, 3, :], xc3(im1, 3))

                    nc.sync.dma_start(
                        j4m[T * 128 * W : (T + 1) * 128 * W, :]
                        .rearrange("(p q) c -> p (q c)", p=128),
                        j4t[:],
                    )

                t00 = th[:, m * 6 + 0 : m * 6 + 1]
                t01 = th[:, m * 6 + 1 : m * 6 + 2]
                t02 = th[:, m * 6 + 2 : m * 6 + 3]
                t10 = th[:, m * 6 + 3 : m * 6 + 4]
                t11 = th[:, m * 6 + 4 : m * 6 + 5]
                t12 = th[:, m * 6 + 5 : m * 6 + 6]
                for T in range(NT):
                    ysT = ys_t[:, T : T + 1]
                    tiny = wpool.tile([128, 2], f32, tag="tiny")
                    # ys*t01, ys*t11
                    nc.vector.tensor_scalar(tiny[:, 0:1], ysT, t01, None, OP.mult)
                    nc.vector.tensor_scalar(tiny[:, 1:2], ysT, t11, None, OP.mult)

                    x = wpool.tile([128, W], f32, tag="x")
                    y = wpool.tile([128, W], f32, tag="y")
                    nc.vector.tensor_scalar(x[:], xs_t[:], t00, None, OP.mult)
                    nc.vector.tensor_scalar(x[:], x[:], tiny[:, 0:1], t02, OP.add, OP.add)
                    nc.vector.tensor_scalar(x[:], x[:], 1.0, float(W) * 0.5, OP.add, OP.mult)
                    nc.vector.tensor_scalar(y[:], xs_t[:], t10, None, OP.mult)
                    nc.vector.tensor_scalar(y[:], y[:], tiny[:, 1:2], t12, OP.add, OP.add)
                    nc.vector.tensor_scalar(y[:], y[:], 1.0, float(H) * 0.5, OP.add, OP.mult)

                    # floors (round-to-nearest-even of v-0.5 == floor except at
                    # exact integers, where bilinear output is unaffected)
                    x0i = wpool.tile([128, W], i32, tag="x0i")
                    y0i = wpool.tile([128, W], i32, tag="y0i")
                    nc.scalar.activation(x0i[:], x[:], AF.Copy, bias=-0.5)
                    nc.scalar.activation(y0i[:], y[:], AF.Copy, bias=-0.5)
                    x0f = wpool.tile([128, W], f32, tag="x0f")
                    y0f = wpool.tile([128, W], f32, tag="y0f")
                    nc.scalar.activation(x0f[:], x0i[:], AF.Copy)
                    nc.scalar.activation(y0f[:], y0i[:], AF.Copy)

                    x0c = wpool.tile([128, W], f32, tag="x0c")
                    x1c = wpool.tile([128, W], f32, tag="x1c")
                    y0c = wpool.tile([128, W], f32, tag="y0c")
                    y1c = wpool.tile([128, W], f32, tag="y1c")
                    nc.vector.tensor_scalar(x0c[:], x0f[:], 0.0, float(W - 1), OP.max, OP.min)
                    nc.vector.tensor_scalar(x1c[:], x0f[:], -1.0, 1.0, OP.max, OP.add)
                    nc.vector.tensor_scalar(x1c[:], x1c[:], float(W - 1), None, OP.min)
                    nc.vector.tensor_scalar(y0c[:], y0f[:], 0.0, float(H - 1), OP.max, OP.min)
                    nc.vector.tensor_scalar(y1c[:], y0f[:], -1.0, 1.0, OP.max, OP.add)
                    nc.vector.tensor_scalar(y1c[:], y1c[:], float(H - 1), None, OP.min)

                    xc = wpool.tile([128, W], f32, tag="xc")
                    yc = wpool.tile([128, W], f32, tag="yc")
                    nc.vector.tensor_scalar(xc[:], x[:], 0.0, float(W - 1), OP.max, OP.min)
                    nc.vector.tensor_scalar(yc[:], y[:], 0.0, float(H - 1), OP.max, OP.min)

                    dxa = wpool.tile([128, W], f32, tag="dxa")
                    dxc = wpool.tile([128, W], f32, tag="dxc")
                    dya = wpool.tile([128, W], f32, tag="dya")
                    dyb = wpool.tile([128, W], f32, tag="dyb")
                    nc.vector.tensor_tensor(dxa[:], x1c[:], xc[:], OP.subtract)
                    nc.vector.tensor_tensor(dxc[:], xc[:], x0c[:], OP.subtract)
                    nc.vector.tensor_tensor(dya[:], y1c[:], yc[:], OP.subtract)
                    nc.vector.tensor_tensor(dyb[:], yc[:], y0c[:], OP.subtract)

                    wa = wpool.tile([128, W], f32, tag="wa")
                    wb = wpool.tile([128, W], f32, tag="wb")
                    wc_ = wpool.tile([128, W], f32, tag="wc")
                    wd = wpool.tile([128, W], f32, tag="wd")
                    nc.vector.tensor_tensor(wa[:], dxa[:], dya[:], OP.mult)
                    nc.vector.tensor_tensor(wb[:], dxa[:], dyb[:], OP.mult)
                    nc.vector.tensor_tensor(wc_[:], dxc[:], dya[:], OP.mult)
                    nc.vector.tensor_tensor(wd[:], dxc[:], dyb[:], OP.mult)

                    # J4 slot index; OOB pixels (all-zero weights) get +BIG so
                    # the gather's bounds check skips their descriptors.
                    vf = wpool.tile([128, W], f32, tag="vf")
                    nc.vector.scalar_tensor_tensor(
                        vf[:], y0c[:], float(W), x0c[:], op0=OP.mult, op1=OP.add
                    )
                    # u = x0f*(382-x0f) >= 0 iff 0 <= x0 <= 382; same for y
                    t1 = wpool.tile([128, W], f32, tag="t1")
                    u = wpool.tile([128, W], f32, tag="u")
                    nc.vector.tensor_scalar(t1[:], x0f[:], float(W - 2), None, OP.subtract)
                    nc.vector.scalar_tensor_tensor(u[:], x0f[:], -1.0, t1[:], op0=OP.mult, op1=OP.mult)
                    t2 = wpool.tile([128, W], f32, tag="t2")
                    v = wpool.tile([128, W], f32, tag="v")
                    nc.vector.tensor_scalar(t2[:], y0f[:], float(H - 2), None, OP.subtract)
                    nc.vector.scalar_tensor_tensor(v[:], y0f[:], -1.0, t2[:], op0=OP.mult, op1=OP.mult)
                    wmin = wpool.tile([128, W], f32, tag="wmin")
                    nc.vector.tensor_tensor(wmin[:], u[:], v[:], OP.min)
                    s = wpool.tile([128, W], f32, tag="s")
                    nc.vector.tensor_scalar(s[:], wmin[:], 0.0, None, OP.is_lt)
                    vf2 = wpool.tile([128, W], f32, tag="vf2")
                    nc.vector.scalar_tensor_tensor(vf2[:], s[:], BIG, vf[:], op0=OP.mult, op1=OP.add)

                    vi = wpool.tile([128, W], i32, tag="vi")
                    nc.vector.tensor_copy(vi[:], vf2[:])

                    g = gpool.tile([128, W * 12], f32, tag="g")
                    for k in range(W):
                        nc.gpsimd.indirect_dma_start(
                            out=g[:, k * 12 : (k + 1) * 12],
                            out_offset=None,
                            in_=j4m[:],
                            in_offset=bass.IndirectOffsetOnAxis(ap=vi[:, k : k + 1], axis=0),
                            bounds_check=NSLOT - 1,
                            oob_is_err=False,
                        )

                    def wbc(t_):
                        return t_[:].rearrange("p (k one) -> p k one", one=1).to_broadcast([128, W, 3])

                    acc = wpool.tile([128, W * 3], f32, tag="acc")
                    tmp = wpool.tile([128, W * 3], f32, tag="tmp")
                    a3 = acc[:].rearrange("p (k c) -> p k c", c=3)
                    t3 = tmp[:].rearrange("p (k c) -> p k c", c=3)
                    g4 = g[:].rearrange("p (k t c) -> p k t c", t=4, c=3)
                    nc.vector.tensor_tensor(a3[:], g4[:, :, 0, :], wbc(wa), OP.mult)
                    nc.vector.tensor_tensor(t3[:], g4[:, :, 1, :], wbc(wc_), OP.mult)
                    nc.vector.tensor_tensor(a3[:], a3[:], t3[:], OP.add)
                    nc.vector.tensor_tensor(t3[:], g4[:, :, 2, :], wbc(wb), OP.mult)
                    nc.vector.tensor_tensor(a3[:], a3[:], t3[:], OP.add)
                    nc.vector.tensor_tensor(t3[:], g4[:, :, 3, :], wbc(wd), OP.mult)
                    nc.vector.tensor_tensor(a3[:], a3[:], t3[:], OP.add)

                    nc.sync.dma_start(
                        dram_ap(out_d, m * IMG_ELEMS + T * 128 * ROW_ELEMS,
                                [[ROW_ELEMS, 128], [1, ROW_ELEMS]]),
                        acc[:],
                    )

    nc.compile()
    return nc


class _Runner:
    def __init__(self, nc, n_cores):
        import jax
        from jax.sharding import Mesh, PartitionSpec
        from jax.experimental.shard_map import shard_map
        import concourse.mybir as mybir
        from concourse.bass2jax import (
            _bass_exec_p, partition_id_tensor, install_neuronx_cc_hook,
        )

        install_neuronx_cc_hook()
        self.jax = jax
        self.n_cores = n_cores
        partition_name = nc.partition_id_tensor.name if nc.partition_id_tensor else None
        in_names, out_names, out_avals, zero_outs = [], [], [], []
        for alloc in nc.m.functions[0].allocations:
            if not isinstance(alloc, mybir.MemoryLocationSet):
                continue
            name = alloc.memorylocations[0].name
            if alloc.kind == "ExternalInput":
                if name != partition_name:
                    in_names.append(name)
            elif alloc.kind == "ExternalOutput":
                shape = tuple(alloc.tensor_shape)
                dtype = mybir.dt.np(alloc.dtype)
                out_avals.append(jax.core.ShapedArray(shape, dtype))
                out_names.append(name)
                zero_outs.append(np.zeros(shape, dtype))
        self.in_names = list(in_names)
        self.out_names = out_names
        self.zero_outs = zero_outs
        n_params = len(in_names)
        n_outs = len(out_names)
        all_in_names = in_names + out_names
        if partition_name is not None:
            all_in_names.append(partition_name)

        def _body(*args):
            operands = list(args)
            if partition_name is not None:
                operands.append(partition_id_tensor())
            outs = _bass_exec_p.bind(
                *operands,
                out_avals=tuple(out_avals),
                in_names=tuple(all_in_names),
                out_names=tuple(out_names),
                lowering_input_output_aliases=(),
                sim_require_finite=False,
                sim_require_nnan=False,
                nc=nc,
            )
            return tuple(outs)

        devices = jax.devices()[:n_cores]
        self.mesh = Mesh(np.asarray(devices), ("core",))
        in_specs = (PartitionSpec("core"),) * (n_params + n_outs)
        out_specs = (PartitionSpec("core"),) * n_outs
        self.fn = jax.jit(
            shard_map(_body, mesh=self.mesh, in_specs=in_specs,
                      out_specs=out_specs, check_rep=False),
            keep_unused=True,
        )

    def run(self, in_maps):
        from jax.sharding import NamedSharding, PartitionSpec
        sharding = NamedSharding(self.mesh, PartitionSpec("core"))
        concat = [
            np.concatenate([np.asarray(m[name]) for m in in_maps], axis=0)
            for name in self.in_names
        ]
        concat += [
            np.zeros((self.n_cores * z.shape[0], *z.shape[1:]), z.dtype)
            for z in self.zero_outs
        ]
        args = [self.jax.device_put(a, sharding) for a in concat]
        outs = self.fn(*args)
        self.jax.block_until_ready(outs)
        res = []
        for c in range(self.n_cores):
            d = {}
            for i, name in enumerate(self.out_names):
                a = np.asarray(outs[i])
                per_core = (self.n_cores, a.shape[0] // self.n_cores) + a.shape[1:]
                d[name] = a.reshape(per_core)[c]
            res.append(d)
        return res


def _get_runner():
    if "runner" not in _CACHE:
        nc = _build_program()
        _CACHE["runner"] = _Runner(nc, NCORES)
    return _CACHE["runner"]


def _host_constants():
    import jax.numpy as jnp

    xs = np.asarray(jnp.linspace(-1.0, 1.0, W, dtype=jnp.float32))
    ys = np.asarray(jnp.linspace(-1.0, 1.0, H, dtype=jnp.float32))
    xs_c = np.tile(xs[None, :], (128, 1)).astype(np.float32)
    # ys_c[p, T] = ys[128*T + p]
    ys_c = ys.reshape(NT, 128).T.copy().astype(np.float32)
    return xs_c, ys_c


def _assign_images(theta):
    """Greedy balance of per-image in-bounds sample counts across cores.

    The gather skips descriptors for out-of-bounds pixels, so per-image
    device cost is proportional to its in-bounds fraction. Estimate it on a
    coarse 64x64 grid and pack images into cores (8 each) greedily.
    Returns perm: perm[c*BPC+s] = original image index placed at core c slot s.
    """
    g = np.linspace(-1.0, 1.0, 64, dtype=np.float32)
    xx, yy = np.meshgrid(g, g)
    coords = np.stack([xx.ravel(), yy.ravel(), np.ones(xx.size, np.float32)], 0)
    t = theta.astype(np.float32) @ coords          # [B, 2, n]
    x = (t[:, 0] + 1.0) * (W * 0.5)
    y = (t[:, 1] + 1.0) * (H * 0.5)
    x0 = np.floor(x)
    y0 = np.floor(y)
    cnt = ((x0 >= 0) & (x0 <= W - 2) & (y0 >= 0) & (y0 <= H - 2)).sum(1)
    order = np.argsort(-cnt, kind="stable")
    loads = np.zeros(NCORES)
    slots = [[] for _ in range(NCORES)]
    for i in order:
        free = [c for c in range(NCORES) if len(slots[c]) < BPC]
        c = min(free, key=lambda c: loads[c])
        slots[c].append(int(i))
        loads[c] += cnt[i]
    return np.array([i for c in range(NCORES) for i in slots[c]], dtype=np.int64)


def build_in_maps(images, theta):
    """Shared by kernel() and any external timing harness."""
    images = np.ascontiguousarray(images, dtype=np.float32)
    theta = np.ascontiguousarray(theta, dtype=np.float32)
    perm = _assign_images(theta)
    xs_c, ys_c = _host_constants()
    in_maps = []
    for c in range(NCORES):
        idx = perm[c * BPC : (c + 1) * BPC]
        imgs = images[idx].reshape(-1)
        imgs = np.concatenate([imgs, np.zeros(1280, np.float32)])
        th = theta[idx].reshape(1, BPC * 6)
        in_maps.append({"images": imgs, "theta": th, "xs_c": xs_c, "ys_c": ys_c})
    return in_maps, perm


def kernel(images, theta):
    images = np.ascontiguousarray(images, dtype=np.float32)
    theta = np.ascontiguousarray(theta, dtype=np.float32)
    assert images.shape == (B, H, W, C) and theta.shape == (B, 2, 3)
    runner = _get_runner()
    in_maps, perm = build_in_maps(images, theta)
    res = runner.run(in_maps)
    out = np.empty((B, H, W, C), np.float32)
    for c in range(NCORES):
        out[perm[c * BPC : (c + 1) * BPC]] = res[c]["out"].reshape(BPC, H, W, C)
    return out
